# revision 6
# baseline (speedup 1.0000x reference)
"""Trainium2 Bass kernel for nn_DeRNN_4054449127979.

Network (per reference):
  stage1: 6 shared-weight single-channel LSTMs (hidden 16) over T=1024,
          folded as one LSTM on [B*6, T, 1]; keep last hidden -> feat [B, 96]
  stage2: LSTM(1 -> 128) over the 96 features as a sequence (return_seq)
  stage3: LSTM(128 -> 128) over those 96 steps; keep last hidden
  head:   relu(fc1) -> relu(fc3) -> fc2  -> [B, 2]

Sharding: pure data parallel over batch across 8 cores (B=2048 -> 256/core).
No collectives (inference, params replicated per core).

v3 design notes (vs the fp32r baseline):
- all matmuls bf16 (psum fp32). fp32r pays 4 cyc/row under 256 free cols;
  bf16 is 1 cyc/row always. Verified numerically: full-bf16 pipeline
  rel err ~6e-4 vs 2e-2 budget.
- stage1 per stream-step is ONE psum bank [96, 4x128]: per gate a [96->96]
  h-matmul plus a [8->96] x/bias matmul whose moving operand is a direct
  view of the DMA-staged x chunk (rows: 6 x channels + 2 ones rows carrying
  bias hi/lo). No per-step x copies on any engine.
- the g-gate weight columns are pre-scaled by 2 so ONE merged Sigmoid over
  the whole bank yields [si, sf, so, s(2g)]; tanh(g) = 2*s(2g)-1 is a cheap
  DVE tensor_scalar. tanh(c) stays on Act. Act: 2 instrs/stream-step.
- sigma outputs are bf16 so DVE tensor ops run in 2x/4x perf modes.
- stage2: bias+x enter via a [3->128] matmul against a staged
  [feat; ones; ones] flat tile. stage3: bias via [2->128] matmul on ones.
  Same merged-sigmoid cell.
"""

import sys

import numpy as np

sys.path.insert(0, "/opt/trn_rl_repo")

import concourse.bass as bass  # noqa: E402
import concourse.tile as tile  # noqa: E402
from concourse import bacc, mybir  # noqa: E402

F32 = mybir.dt.float32
BF16 = mybir.dt.bfloat16
AF = mybir.ActivationFunctionType
ALU = mybir.AluOpType

B = 2048
NCORES = 8
BC = B // NCORES  # 256
BH = BC // 2  # 128 per stream
NCH = 6
H1 = 16
G1 = NCH * H1  # 96
H2 = 128
T1_FULL = 1024
T2_FULL = 96
CH = 32  # stage-1 steps per staged x chunk
XR = 8  # staged x rows: 6 channels + 2 ones (bias hi/lo)

# gate column order in psum banks: i, f, o, g(x2)
# torch weight row bases (order i, f, g, o): i=0, f=H, g=2H, o=3H
TORCH_BASE_1 = (0, H1, 3 * H1, 2 * H1)  # for (i, f, o, g)
TORCH_BASE_2 = (0, H2, 3 * H2, 2 * H2)

# packed weight column map (bf16)
_off = 0


def _take(n):
    global _off
    o = _off
    _off += n
    return o


W1H_O = _take(4 * G1)  # [0:96] stage1 h-part, per gate 96 cols
W1XO_O = _take(4 * G1)  # [0:8]  stage1 x + bias rows, per gate 96 cols
W2H_O = _take(4 * H2)  # [0:128]
W2XB_O = _take(4 * H2)  # [0:3] rows: b2_hi, b2_lo, w2x
W3X_O = _take(4 * H2)
W3H_O = _take(4 * H2)
B3_O = _take(4 * H2)  # [0:2] rows: b3_hi, b3_lo
WF1_O = _take(H2)
WF3_O = _take(H2)
WF2_O = _take(2)
BF1_O = _take(2)  # f32 biases stored as 2 bf16 cols (bitcast)
BF3_O = _take(2)
BF2_O = _take(2)
WCOLS = _off


def _r(ap, pattern, **kw):
    return ap.rearrange(pattern, **kw)


def build_program(T1=T1_FULL, T2=T2_FULL, ch=CH, staggered=False):
    assert T1 % (2 * ch) == 0
    nc = bacc.Bacc("TRN2", target_bir_lowering=False)

    tpad = 2 * ch
    x_t = nc.declare_dram_parameter("xT", [XR, T1 + tpad, BC], BF16, isOutput=False)
    wp_d = nc.declare_dram_parameter("wpack", [128, WCOLS], BF16, isOutput=False)
    y_t = nc.declare_dram_parameter("yT", [2, BC], F32, isOutput=True)

    feat_d = nc.dram_tensor("featstage", [G1, BC], BF16)

    def mm(out, lhsT, rhs, start, stop):
        nc.tensor.matmul(out, lhsT, rhs, start=start, stop=stop)

    with tile.TileContext(nc) as tc:
        with (
            tc.tile_pool(name="wpool", bufs=1) as wpool,
            tc.tile_pool(name="state", bufs=1) as state,
            tc.tile_pool(name="work", bufs=3) as work,
        ):
            wp = wpool.tile([128, WCOLS], BF16)
            nc.sync.dma_start(wp[:], wp_d[:])

            w1h = [wp[0:G1, W1H_O + G1 * t : W1H_O + G1 * (t + 1)] for t in range(4)]
            w1xo = [wp[0:XR, W1XO_O + G1 * t : W1XO_O + G1 * (t + 1)] for t in range(4)]
            w2h = [wp[0:H2, W2H_O + H2 * t : W2H_O + H2 * (t + 1)] for t in range(4)]
            w2xb = [wp[0:3, W2XB_O + H2 * t : W2XB_O + H2 * (t + 1)] for t in range(4)]
            w3x = [wp[0:H2, W3X_O + H2 * t : W3X_O + H2 * (t + 1)] for t in range(4)]
            w3h = [wp[0:H2, W3H_O + H2 * t : W3H_O + H2 * (t + 1)] for t in range(4)]
            b3w = [wp[0:2, B3_O + H2 * t : B3_O + H2 * (t + 1)] for t in range(4)]
            wf1 = wp[0:H2, WF1_O : WF1_O + H2]
            wf3 = wp[0:H2, WF3_O : WF3_O + H2]
            wf2 = wp[0:H2, WF2_O : WF2_O + 2]
            bf1 = wp[0:H2, BF1_O : BF1_O + 2].bitcast(F32)
            bf3 = wp[0:H2, BF3_O : BF3_O + 2].bitcast(F32)
            bf2 = wp[0:2, BF2_O : BF2_O + 2].bitcast(F32)

            # ---- stage 1: two interleaved batch half-streams ----
            st = [
                [state.tile([G1, BH], BF16, name=f"st{s}_{p}") for p in range(2)]
                for s in range(2)
            ]
            c1 = [state.tile([G1, BH], F32, name=f"c1_{s}") for s in range(2)]
            for s in range(2):
                for p in range(2):
                    nc.vector.memset(st[s][p][:], 0.0)
                nc.vector.memset(c1[s][:], 0.0)

            with (
                tc.tile_pool(name="ps1pool", bufs=2, space="PSUM") as ps1pool,
                tc.tile_pool(name="xsb", bufs=1) as xsb,
            ):
                xstage = [
                    xsb.tile([XR, ch * BC], BF16, name=f"xst_{k}") for k in range(2)
                ]
                xv = _r(x_t[:], "c t b -> c (t b)")
                nc.sync.dma_start(xstage[0][:], xv[:, 0 : ch * BC])
                nc.sync.dma_start(xstage[1][:], xv[:, ch * BC : 2 * ch * BC])
                nchunks = T1 // ch

                def xview(k, jj, s):
                    return _r(xstage[k], "p (t b) -> p t b", b=BC)[
                        :, jj, s * BH : (s + 1) * BH
                    ]

                def s1_mm(s, p, k, jj):
                    ps = ps1pool.tile([G1, 512], F32, name=f"ps1_{s}", tag=f"ps1_{s}")
                    rhs_h = st[s][p][:]
                    rhs_x = xview(k, jj, s)
                    for t in range(4):
                        reg = ps[:, BH * t : BH * (t + 1)]
                        mm(reg, w1h[t], rhs_h, True, False)
                        mm(reg, w1xo[t], rhs_x, False, True)
                    return ps

                def s1_cell(s, p, ps):
                    sS = work.tile([G1, 512], BF16, name=f"S{s}", tag=f"S{s}")
                    nc.scalar.activation(sS[:], ps[:], AF.Sigmoid)
                    si = sS[:, 0:BH]
                    sf = sS[:, BH : 2 * BH]
                    so = sS[:, 2 * BH : 3 * BH]
                    sg = sS[:, 3 * BH : 4 * BH]
                    gt = work.tile([G1, BH], BF16, name=f"gt{s}", tag=f"gt{s}")
                    nc.vector.tensor_scalar(gt[:], sg, 2.0, 1.0, ALU.mult, ALU.subtract)
                    u = work.tile([G1, BH], BF16, name=f"u{s}", tag=f"u{s}")
                    nc.vector.tensor_mul(u[:], gt[:], si)
                    fc = work.tile([G1, BH], F32, name=f"fc{s}", tag=f"fc{s}")
                    nc.gpsimd.tensor_mul(fc[:], sf, c1[s][:])
                    nc.vector.tensor_add(c1[s][:], u[:], fc[:])
                    th = work.tile([G1, BH], BF16, name=f"th{s}", tag=f"th{s}")
                    nc.scalar.activation(th[:], c1[s][:], AF.Tanh)
                    nc.vector.tensor_mul(st[s][1 - p][:], so, th[:])

                # software-pipelined: stream B runs half a step behind A, so
                # B's matmuls overlap A's cell math and vice versa.
                def chunk_pair(ivc):
                    pendA = (0, s1_mm(0, 0, 0, 0))
                    for k in range(2):
                        for jj in range(ch):
                            if jj < ch - 1:
                                kn, jn = k, jj + 1
                            else:
                                kn, jn = 1 - k, 0
                            p = jj % 2
                            psB = s1_mm(1, p, k, jj)
                            ap, aps = pendA
                            s1_cell(0, ap, aps)
                            if not (k == 1 and jj == ch - 1):
                                pendA = (1 - p, s1_mm(0, 1 - p, kn, jn))
                            s1_cell(1, p, psB)
                        pre = (ivc + (2 + k)) * (ch * BC)
                        nc.sync.dma_start(
                            xstage[k][:], xv[:, bass.ds(pre, ch * BC)]
                        )

                chunk_pair(0)  # peeled: absorbs prologue DMA waits
                assert nchunks >= 4
                with tc.For_i(2, nchunks, 2, staggered_reset=staggered) as ivc:
                    chunk_pair(ivc)

                # final h -> feat  (last write was into ping T1%2)
                pf = T1 % 2
                nc.sync.dma_start(feat_d[:, 0:BH], st[0][pf][:])
                nc.sync.dma_start(feat_d[:, BH:BC], st[1][pf][:])

            # ---- stages 2 & 3, wavefronted ----
            with tc.tile_pool(name="psum", bufs=2, space="PSUM") as psum_pool:
                h2s = [state.tile([H2, BC], BF16, name=f"h2s_{p}") for p in range(2)]
                c2 = state.tile([H2, BC], F32)
                h3s = [state.tile([H2, BC], BF16, name=f"h3s_{p}") for p in range(2)]
                c3 = state.tile([H2, BC], F32)
                for t_ in h2s + h3s:
                    nc.vector.memset(t_[:], 0.0)
                nc.vector.memset(c2[:], 0.0)
                nc.vector.memset(c3[:], 0.0)
                # [ones; ones; feat] staged flat for the stage2 x/bias matmul
                # (ones first so step3's bias matmul rhs starts at partition 0)
                fo2 = state.tile([3, T2 * BC], BF16, name="fo2")
                nc.vector.memset(fo2[0:2, :], 1.0)
                nc.sync.dma_start(fo2[2:3, :], _r(feat_d[:], "r b -> (r b)"))

                def cell23(ps, c, h_out, pfx):
                    sS = work.tile([H2, 1024], BF16, name=f"S{pfx}", tag="S23")
                    nc.scalar.activation(sS[:], ps[:], AF.Sigmoid)
                    si = sS[:, 0:BC]
                    sf = sS[:, BC : 2 * BC]
                    so = sS[:, 2 * BC : 3 * BC]
                    sg = sS[:, 3 * BC : 4 * BC]
                    gt = work.tile([H2, BC], BF16, name=f"gt{pfx}", tag="gt23")
                    nc.vector.tensor_scalar(gt[:], sg, 2.0, 1.0, ALU.mult, ALU.subtract)
                    u = work.tile([H2, BC], BF16, name=f"u{pfx}", tag="u23")
                    nc.vector.tensor_mul(u[:], gt[:], si)
                    fc = work.tile([H2, BC], F32, name=f"fc{pfx}", tag="fc23")
                    nc.gpsimd.tensor_mul(fc[:], sf, c[:])
                    nc.vector.tensor_add(c[:], u[:], fc[:])
                    th = work.tile([H2, BC], BF16, name=f"th{pfx}", tag="th23")
                    nc.scalar.activation(th[:], c[:], AF.Tanh)
                    nc.vector.tensor_mul(h_out[:], so, th[:])

                def step2(j, xoff):
                    ps = psum_pool.tile([128, 1024], F32, name="ps2", tag="ps2")
                    rhs_h = h2s[j % 2][:]
                    rhs_x = fo2[0:3, bass.ds(xoff, BC)]
                    for t in range(4):
                        reg = ps[:, BC * t : BC * (t + 1)]
                        mm(reg, w2h[t], rhs_h, True, False)
                        mm(reg, w2xb[t], rhs_x, False, True)
                    cell23(ps, c2, h2s[(j + 1) % 2], "2")

                def step3(j):
                    ps = psum_pool.tile([128, 1024], F32, name="ps3", tag="ps3")
                    rhs_x = h2s[(j + 1) % 2][:]
                    rhs_h = h3s[j % 2][:]
                    rhs_b = fo2[0:2, 0:BC]
                    for t in range(4):
                        reg = ps[:, BC * t : BC * (t + 1)]
                        mm(reg, w3x[t], rhs_x, True, False)
                        mm(reg, w3h[t], rhs_h, False, False)
                        mm(reg, b3w[t], rhs_b, False, True)
                    cell23(ps, c3, h3s[(j + 1) % 2], "3")

                W2 = 16

                def wave_block(ivw):
                    for jj in range(W2):
                        j = jj  # parity is static; absolute offset via ivw
                        step2(j, ivw * BC + jj * BC)
                        step3(j)

                wave_block(0)  # peeled
                with tc.For_i(W2, T2, W2, staggered_reset=staggered) as ivw:
                    wave_block(ivw)

                # ---- FC head ----
                h3f = h3s[T2 % 2]
                psf = psum_pool.tile([128, 1024], F32, name="psf", tag="ps2")
                mm(psf[:, 0:BC], wf1, h3f[:], True, True)
                a1 = work.tile([H2, BC], BF16)
                nc.vector.tensor_scalar(
                    a1[:], psf[:, 0:BC], bf1, 0.0, op0=ALU.add, op1=ALU.max
                )
                mm(psf[:, 512 : 512 + BC], wf3, a1[:], True, True)
                a3 = work.tile([H2, BC], BF16)
                nc.vector.tensor_scalar(
                    a3[:], psf[:, 512 : 512 + BC], bf3, 0.0, op0=ALU.add, op1=ALU.max
                )
                mm(psf[0:2, 768 : 768 + BC], wf2, a3[:], True, True)
                yt = work.tile([2, BC], F32)
                nc.vector.tensor_scalar_add(yt[:], psf[0:2, 768 : 768 + BC], bf2)
                nc.sync.dma_start(y_t[:], yt[:])

    nc.compile()
    return nc


def _bf16(a):
    import ml_dtypes

    return np.asarray(a, np.float32).astype(ml_dtypes.bfloat16)


def _hi_lo(v):
    hi = _bf16(v)
    lo = _bf16(np.asarray(v, np.float32) - hi.astype(np.float32))
    return hi.astype(np.float32), lo.astype(np.float32)


def pack_weights(i):
    f32 = np.float32
    wp = np.zeros((128, WCOLS), f32)
    Wih, Whh = np.asarray(i["rnn_Wih"], f32), np.asarray(i["rnn_Whh"], f32)
    bb1 = np.asarray(i["rnn_bih"], f32) + np.asarray(i["rnn_bhh"], f32)
    for t, base in enumerate(TORCH_BASE_1):
        sc = 2.0 if t == 3 else 1.0  # g-gate columns pre-scaled for 2*sig(2g)-1
        o = W1H_O + G1 * t
        for c in range(NCH):
            wp[16 * c : 16 * c + 16, o + 16 * c : o + 16 * c + 16] = (
                sc * Whh[base : base + H1, :].T
            )
        o = W1XO_O + G1 * t
        for c in range(NCH):
            wp[c, o + 16 * c : o + 16 * c + 16] = sc * Wih[base : base + H1, 0]
        bhi, blo = _hi_lo(sc * bb1[base : base + H1])
        wp[NCH, o : o + G1] = np.tile(bhi, NCH)
        wp[NCH + 1, o : o + G1] = np.tile(blo, NCH)
    bb2 = np.asarray(i["rnn2_bih0"], f32) + np.asarray(i["rnn2_bhh0"], f32)
    for t, base in enumerate(TORCH_BASE_2):
        sc = 2.0 if t == 3 else 1.0
        wp[0:H2, W2H_O + H2 * t : W2H_O + H2 * (t + 1)] = (
            sc * np.asarray(i["rnn2_Whh0"], f32)[base : base + H2, :].T
        )
        o = W2XB_O + H2 * t
        bhi, blo = _hi_lo(sc * bb2[base : base + H2])
        wp[0, o : o + H2] = bhi
        wp[1, o : o + H2] = blo
        wp[2, o : o + H2] = sc * np.asarray(i["rnn2_Wih0"], f32)[base : base + H2, 0]
    bb3 = np.asarray(i["rnn2_bih1"], f32) + np.asarray(i["rnn2_bhh1"], f32)
    for t, base in enumerate(TORCH_BASE_2):
        sc = 2.0 if t == 3 else 1.0
        wp[0:H2, W3X_O + H2 * t : W3X_O + H2 * (t + 1)] = (
            sc * np.asarray(i["rnn2_Wih1"], f32)[base : base + H2, :].T
        )
        wp[0:H2, W3H_O + H2 * t : W3H_O + H2 * (t + 1)] = (
            sc * np.asarray(i["rnn2_Whh1"], f32)[base : base + H2, :].T
        )
        bhi, blo = _hi_lo(sc * bb3[base : base + H2])
        wp[0, B3_O + H2 * t : B3_O + H2 * (t + 1)] = bhi
        wp[1, B3_O + H2 * t : B3_O + H2 * (t + 1)] = blo
    wp[0:H2, WF1_O : WF1_O + H2] = np.asarray(i["fc1_W"], f32).T
    wp[0:H2, WF3_O : WF3_O + H2] = np.asarray(i["fc3_W"], f32).T
    wp[0:H2, WF2_O : WF2_O + 2] = np.asarray(i["fc2_W"], f32).T
    wpb = _bf16(wp)
    # f32 biases stored exactly as 2 bf16 columns (bitcast on device)
    import ml_dtypes

    def put_f32(col, vals, n):
        raw = np.asarray(vals, np.float32).view(np.uint16).reshape(n, 2)
        wpb[0:n, col : col + 2] = raw.view(ml_dtypes.bfloat16)

    put_f32(BF1_O, i["fc1_b"], H2)
    put_f32(BF3_O, i["fc3_b"], H2)
    put_f32(BF2_O, i["fc2_b"], 2)
    return wpb


def make_in_maps(inputs, T1=T1_FULL, ch=CH):
    wp = pack_weights(inputs)
    x = np.asarray(inputs["x"], np.float32)
    tpad = 2 * ch
    maps = []
    for k in range(NCORES):
        xk = np.zeros((XR, T1 + tpad, BC), np.float32)
        xk[0:NCH, :T1, :] = np.ascontiguousarray(
            x[k * BC : (k + 1) * BC, :T1, :].transpose(2, 1, 0)
        )
        xk[NCH : NCH + 2, :, :] = 1.0
        maps.append({"xT": _bf16(xk), "wpack": wp})
    return maps


def kernel(**inputs):
    from concourse.bass_utils import run_bass_kernel_spmd

    nc = build_program()
    in_maps = make_in_maps(inputs)
    res = run_bass_kernel_spmd(nc, in_maps, list(range(NCORES)))
    outs = [np.asarray(res.results[k]["yT"]) for k in range(NCORES)]
    return np.concatenate([o.T for o in outs], axis=0).astype(np.float32)
